# revision 95
# baseline (speedup 1.0000x reference)
"""Trainium2 Bass kernel for a 2-layer GCN link predictor (NetLinkTrain).

Math: z = relu(A @ (x @ W1)); z2 = A @ (z @ W2); out = [z2[e0], z2[e1]] @ Wlin.T
where A = D^-1/2 (Adj + I) D^-1/2.

Since there is no nonlinearity after conv2, fold W2 and Wlin:
  W2' = W2 @ [Wlin[:, :128].T | Wlin[:, 128:].T]   (shape [128, 4])
  c   = A @ (z @ W2')                              (shape [N, 4])
  out[k, j] = c[e0_k, j] + c[e1_k, 2 + j]

Sharding: edges are sharded by destination-node range (core c owns nodes
[c*6250, (c+1)*6250)); each core fully owns its segment sums, so the only
communication is two AllGathers (1.6MB shard each for the y and c tables).

Per the edge-parallel sharding strategy, each core's input shard includes its
gathered L1 messages: x2p[i] holds the x rows of the core's edges in packed
slot order (pairs of rows per 512B line), so the L1 "gather" is a contiguous
DRAM->SBUF stream at full DMA rate -- no per-edge descriptors.  Slot layout
in SBUF is identical to a dma_gather result: slot s = chunk*128 + p sits at
partition p, column block chunk; DRAM row r of a 256-slot block holds slot
(r//2 % 128) of half r%2 (so each 512B line carries two slot rows).

Per core (tables bf16; L2/decode gathers use 256B elements):
  L1: stream x2p group-by-group -> one-hot (iota==dst_local)*norm on DVE
      (tensor_scalar, 4x DVE mode) -> TensorE scatter matmul accumulating
      u^T per 128-dst tile in PSUM.  The 50k appended self-loops are handled
      as a dense per-tile diagonal matmul (lhsT=x_local_tile,
      rhs=diag(dinv^2)) fed by one contiguous DMA of the core's own
      (host-pre-transposed) x rows.
      -> v^T = W1^T u^T -> relu -> z^T -> y = z @ W2' -> y[NT*4] bf16
  y/c tables: one 256B-stride row per NODE (gather elements must be
      256B-aligned), row index (p, t)-major within each A/B sub-table, but
      only the 4 bf16 values are written per row (8B runs at the 7ns
      descriptor floor); the remaining 248B are never read by any consumer.
  L2: dma_gather y_full[row(src)] -> scatter matmul with 4-wide rhs into a
      [128, 4] PSUM tile -> c_tile = dinv2 * y_tile + c_psum in one fused
      DVE op -> c [NT*4] bf16.  Edge slots are sorted into 2 sections per
      group by the A/B sub-table of src, so L2 needs only 2 gather calls per
      group, with exact (unpadded) num_idxs.
  Decode: gather c elements for e0/e1 and add column slices: one DVE
      tensor_tensor per chunk-group slice.  Eval edges are host-sorted into
      4 runs by (e0 in B?, e1 in B?) so each gather call indexes one A/B
      sub-table; within a run, edges with adjacent e0 rows share one 512B
      descriptor (elem_step=row), then edges with adjacent e1 rows among the
      rest, cutting decode descriptors ~25%.  The host un-permutes the
      output rows.

The y/c tables are split into A/B sub-tables by local node tile (< 32 = A)
so that (a) every sub-table row fits int16 (A tops out at exactly 32767) and
(b) each AllGather becomes two contiguous-buffer halves whose first half only
depends on tiles 0-31 -- it overlaps the tail of the producing phase, letting
the L2 A-section gathers (65% of rows) start mid-Layer-1.  Edges sort into 2 sections (A/B of
src); within each (group, section) edges pack densely into 128-slot chunks
(ceil-padding only at section boundaries; groups pad to an even chunk count
for the 512B pair lines) and a chunk spanning a tile boundary is consumed
once per overlapping tile ("use") with foreign edges zeroed in that use's
one-hot column.  Host does index preprocessing and layout only: degree/norm,
sorting, packed/wrapped index layouts, bf16 casts, per-core slot-ordered x
copies.  All FLOPs over node/edge features run on device.
"""

import math
import os
import sys

import numpy as np
import ml_dtypes

sys.path.insert(0, "/opt/trn_rl_repo")

import bass_rust
import concourse.bacc as bacc
import concourse.bass as bass
import concourse.tile as tile
from concourse import mybir
from concourse.bass_utils import run_bass_kernel_spmd

N = 50000
H = 128
P = 128
NC = 8
NPC = N // NC            # 6250 nodes per core
NT = math.ceil(NPC / P)  # 49 dst tiles per core
PADN = NT * P            # 6272 padded nodes per core
NPAD = PADN - NPC        # 22
TA = 32                  # A sub-table: tiles 0..31 (rows max 32767); B: 32..48
AB0 = TA * P             # 3072 rows per core in the A sub-table
AB1 = PADN - AB0         # 3200 rows per core in the B sub-table
NE_EVAL = 200000
EV_PC = NE_EVAL // NC    # 25000 eval edges per core
TG = 5                   # tiles per L1/L2 group
DEC_GROUP = 50           # decode chunks per gather call
PK = 32                  # nodes-per-... 256B element = 32 replicas of 4 vals

F32 = mybir.dt.float32
BF16 = mybir.dt.bfloat16
I16 = mybir.dt.int16

BF = ml_dtypes.bfloat16


def _abrow(n):
    """(is_B, row) of node n in the (p, t)-major split y/c sub-tables.

    Node n = c*NPC + off, off = t*128 + p.
    A (t < 32):  row = c*4096 + p*32 + t        (max 32767)
    B (t >= 32): row = c*2176 + p*17 + (t-32)   (max 17407)
    Both fit int16 with no offset subtraction at gather time.
    """
    n = np.asarray(n)
    c = n // NPC
    off = n - c * NPC
    t = off // P
    p = off % P
    return (t >= TA).astype(np.int64), np.where(
        t < TA, c * AB0 + p * TA + t, c * AB1 + p * (NT - TA) + (t - TA)
    )


def _wrap_idx(v, n_chunks):
    """v: [n_chunks, 128] int -> dma_gather wrapped idx layout [128, n_chunks*8].

    Position i in a call maps to idx[i % 16, base + i // 16]; with p = q*16 + r
    inside chunk j this is row r, col j*8 + q. Rows 16..127 replicate 0..15.
    """
    a16 = v.reshape(n_chunks, 8, 16).transpose(2, 0, 1).reshape(16, n_chunks * 8)
    return np.tile(a16, (8, 1)).astype(np.int16)


def _preprocess(x, edge_index, pos_edge_index, neg_edge_index):
    # degrees INCLUDE the self loop of every node (reference appends them);
    # the appended loops themselves are handled by the dense diagonal term.
    src = np.asarray(edge_index[0]).astype(np.int64)
    dst = np.asarray(edge_index[1]).astype(np.int64)
    deg = (np.bincount(dst, minlength=N) + 1).astype(np.float32)
    dinv = 1.0 / np.sqrt(deg)
    norm = (dinv[src] * dinv[dst]).astype(np.float32)

    core = dst // NPC
    dl = dst - core * NPC
    tl = dl // P
    dloc = (dl % P).astype(np.int32)
    # section: A/B sub-table of src (for the L2 gather's int16 row index)
    sec = (((src % NPC) // P) >= TA).astype(np.int64)

    key = (core * 2 + sec) * NT + tl
    order = np.argsort(key, kind="stable")
    s_src = src[order].astype(np.int32)
    s_dloc = dloc[order]
    s_norm = norm[order]
    s_key = key[order]

    counts = np.bincount(key, minlength=NC * 2 * NT).reshape(NC, 2, NT)

    # Dense chunk packing: within each (group, section) the edges of the
    # group's tiles are packed back-to-back (sorted by tile); only the
    # group-section boundary pads to a 128 multiple, and each group pads to
    # an even total chunk count (512B x2p pair lines).  A chunk that spans a
    # tile boundary is consumed once per overlapping tile ("use"), with the
    # other tiles' slots zeroed in that use's one-hot column.
    NG = math.ceil(NT / TG)
    csum = np.concatenate(
        [np.zeros((NC, 2, 1), np.int64), np.cumsum(counts, axis=2)], axis=2
    )  # per-core cumulative edges before tile t within section s
    gbase = []       # [NG+1] global chunk base per group
    secbase = []     # [NG][2] chunk base per (group, section)
    secn = []        # [NG][2] chunk count per (group, section)
    exactn = []      # [NG][2] exact (max-over-core) edge count per section
    uses = []        # [NG] list of (global chunk, tile)
    tile_uses = []   # [NG] dict tile -> list of local use indices
    tstart = np.zeros((NC, 2, NT), np.int64)  # slot offset of tile run
    pos = 0
    for g in range(NG):
        gbase.append(pos)
        ts = list(range(g * TG, min((g + 1) * TG, NT)))
        sb = []
        sn = []
        en = []
        u = []
        tu = {t: [] for t in ts}
        for s in range(2):
            cnt_c = csum[:, s, ts[-1] + 1] - csum[:, s, ts[0]]
            n = int((-(-cnt_c // P)).max())
            sb.append(pos)
            en.append(int(cnt_c.max()))
            for c in range(NC):
                for t in ts:
                    tstart[c, s, t] = pos * P + (csum[c, s, t] - csum[c, s, ts[0]])
            for t in ts:
                j0 = min(int(tstart[c, s, t]) // P for c in range(NC))
                j1 = max(
                    -(-(int(tstart[c, s, t]) + int(counts[c, s, t])) // P)
                    for c in range(NC)
                )
                j1 = max(j1, j0 + (counts[:, s, t].max() > 0))
                for j in range(j0, j1):
                    tu[t].append(len(u))
                    u.append((j, t))
            pos += n
            sn.append(n)
        if (pos - gbase[g]) % 2:
            pos += 1  # all-pad chunk: no uses, excluded by exact num_idxs
            sn[1] += 1
        secbase.append(sb)
        secn.append(sn)
        exactn.append(en)
        uses.append(u)
        tile_uses.append(tu)
    gbase.append(pos)
    NCH = int(pos)
    NUSE = sum(len(u) for u in uses)

    # per-edge destination slot: tstart of its (core, sec, tile) + rank
    group_start = np.concatenate([[0], np.cumsum(counts.reshape(-1))])[:-1]
    rank = np.arange(len(s_src)) - group_start[s_key]
    g_core = s_key // (2 * NT)
    g_sec = (s_key // NT) % 2
    g_tile = s_key % NT
    dest = tstart[g_core, g_sec, g_tile] + rank

    # global use index map
    use_base = []
    ub = 0
    for g in range(NG):
        use_base.append(ub)
        ub += len(uses[g])
    use_of = {}
    for g in range(NG):
        for i, (j, t) in enumerate(uses[g]):
            use_of[(j, t)] = use_base[g] + i

    xb = np.asarray(x, np.float32).astype(BF)

    per_core = []
    for c in range(NC):
        m = g_core == c
        slot_src = np.zeros(NCH * P, np.int32)
        d = dest[m].astype(np.int64)
        slot_src[d] = s_src[m]

        # one-hot metadata lives per USE: zero except this use's tile's edges
        dstl_u = np.full((NUSE, P), 200, np.float32)
        norm_u = np.zeros((NUSE, P), np.float32)
        e_chunk = d // P
        e_part = d % P
        e_use = np.array(
            [use_of[(int(j), int(t))] for j, t in zip(e_chunk, g_tile[m])],
            np.int64,
        )
        dstl_u[e_use, e_part] = s_dloc[m]
        norm_u[e_use, e_part] = s_norm[m]

        # x2p: the core's gathered L1 message rows in packed slot order.
        # DRAM row r of 256-slot block b holds slot b*256 + (r%256)//2 of
        # half r%2, i.e. each 512B pair line holds slots (q, 128+q).
        sv = slot_src.reshape(-1, 2, P).transpose(0, 2, 1).reshape(-1)
        x2p = np.ascontiguousarray(xb[sv]).reshape(NCH * 64, 2 * H)
        idx2 = _abrow(slot_src.reshape(NCH, P))[1]
        per_core.append(
            dict(
                x2p=x2p,
                idx2=_wrap_idx(idx2, NCH),
                dstl=dstl_u.T.copy(),
                normv=norm_u.T.copy(),
            )
        )

    for c in range(NC):
        # self-loop diagonal metadata: dinv^2 of local node (t, p); 0 beyond NPC
        d2 = np.zeros((NT, P), np.float32)
        d2.reshape(-1)[:NPC] = dinv[c * NPC : (c + 1) * NPC] ** 2
        per_core[c]["dinv2"] = d2.T.copy()
        # local x rows, pre-transposed: xloc[p, t*H:(t+1)*H] = x[c*NPC+t*128+p]
        xl = np.zeros((NT, P, H), BF)
        xl.reshape(-1, H)[:NPC] = xb[c * NPC : (c + 1) * NPC]
        per_core[c]["xloc"] = xl.transpose(1, 0, 2).reshape(P, NT * H).copy()

    # decode metadata: eval edges sorted into 4 runs by (row(e0) in B,
    # row(e1) in B) so each gather call uses one int16-indexed sub-table.
    # Chunk-major slot assignment: slot s of run r -> chunk rbase[r] + s//128,
    # part s%128, so real slots form a call-position prefix (exact num_idxs).
    e0 = np.concatenate([np.asarray(pos_edge_index[0]), np.asarray(neg_edge_index[0])])
    e1 = np.concatenate([np.asarray(pos_edge_index[1]), np.asarray(neg_edge_index[1])])
    b0, r0 = _abrow(e0)
    b1, _ = _abrow(e1)
    dkey = b0 * 2 + b1
    # Within each run, eval edges sort by row(e0); adjacent-row pairs share
    # one 512B descriptor (elem_step=H, elem_size=2H fetches rows R, R+1).
    # Run layout: [2*Dp paired chunks: pair i -> chunks rb+2*(i//128)+h,
    # part i%128, h=0 low row / h=1 high row][Ds single chunks, chunk-major].
    def _adjpairs(cand, rows):
        # greedy adjacent-row pairing over rows[cand] (cand pre-sorted)
        pr, sg, i = [], [], 0
        while i < len(cand):
            if i + 1 < len(cand) and rows[cand[i + 1]] == rows[cand[i]] + 1:
                pr.append((cand[i], cand[i + 1]))
                i += 2
            else:
                sg.append(cand[i])
                i += 1
        return pr, sg

    r1 = _abrow(e1)[1]
    core_pairs = []    # [NC][4] e0-paired (low, high) local eval rows
    core_pairs1 = []   # [NC][4] e1-paired among the e0-singles
    core_singles = []  # [NC][4] leftover local eval rows
    for c in range(NC):
        sl = slice(c * EV_PC, (c + 1) * EV_PC)
        rl0 = r0[sl]
        rl1 = r1[sl]
        kl = dkey[sl]
        prs, prs1, sgs = [], [], []
        for r in range(4):
            idxs = np.where(kl == r)[0]
            o = idxs[np.argsort(rl0[idxs], kind="stable")]
            pr, rest = _adjpairs(o, rl0)
            rest = np.array(rest, np.int64)
            o1 = rest[np.argsort(rl1[rest], kind="stable")] if len(rest) else rest
            pr1, sg = _adjpairs(o1, rl1)
            prs.append(pr)
            prs1.append(pr1)
            sgs.append(sg)
        core_pairs.append(prs)
        core_pairs1.append(prs1)
        core_singles.append(sgs)

    def _cover(npair):
        K, rem = npair // P, npair % P
        return (2 * K + 1) * P + rem if rem else 2 * K * P

    PEX = [max(len(core_pairs[c][r]) for c in range(NC)) for r in range(4)]
    PEX1 = [max(len(core_pairs1[c][r]) for c in range(NC)) for r in range(4)]
    SEX = [max(len(core_singles[c][r]) for c in range(NC)) for r in range(4)]
    DP = [-(-PEX[r] // P) for r in range(4)]    # e0-paired desc-chunks
    DP1 = [-(-PEX1[r] // P) for r in range(4)]  # e1-paired desc-chunks
    DS = [-(-SEX[r] // P) for r in range(4)]    # single chunks per run
    DCR = []
    for r in range(4):
        n = 2 * DP[r] + 2 * DP1[r] + DS[r]
        if n % 2:
            DS[r] += 1
            n += 1
        DCR.append(n)
    DCP = sum(DCR)
    rbase = np.concatenate([[0], np.cumsum(DCR)])
    rbP = np.concatenate([[0], np.cumsum(DP)])
    rbP1 = rbP[-1] + np.concatenate([[0], np.cumsum(DP1)])
    DPT = int(rbP1[-1])
    # per-slot exact cover of a paired region (filled slots are not a strict
    # chunk-major prefix there: chunks 2K and 2K+1 fill parts < rem)
    E1C = [max(_cover(len(core_pairs[c][r])) for c in range(NC))
           for r in range(4)]
    E0C1 = [max(_cover(len(core_pairs1[c][r])) for c in range(NC))
            for r in range(4)]
    for c in range(NC):
        sl = slice(c * EV_PC, (c + 1) * EV_PC)
        v0 = np.zeros(DCP * P, np.int64)
        v1 = np.zeros(DCP * P, np.int64)
        perm = np.full(DCP * P, -1, np.int64)  # chunk-slot -> eval row in [sl]
        pv = np.zeros((DPT, P), np.int64)      # paired idx: low row per desc
        for r in range(4):
            rb = rbase[r]
            for i, (lo, hi) in enumerate(core_pairs[c][r]):
                ci, qi = i // P, i % P
                pv[rbP[r] + ci, qi] = r0[sl][lo]
                for h, ev in ((0, lo), (1, hi)):
                    ch = rb + 2 * ci + h
                    pos = qi * DCP + ch
                    v1[pos] = e1[sl][ev]
                    perm[pos] = ev
            m0 = rb + 2 * DP[r]
            for i, (lo, hi) in enumerate(core_pairs1[c][r]):
                ci, qi = i // P, i % P
                pv[rbP1[r] + ci, qi] = r1[sl][lo]
                for h, ev in ((0, lo), (1, hi)):
                    ch = m0 + 2 * ci + h
                    pos = qi * DCP + ch
                    v0[pos] = e0[sl][ev]
                    perm[pos] = ev
            sb_ = m0 + 2 * DP1[r]
            for s, ev in enumerate(core_singles[c][r]):
                ch = sb_ + s // P
                pos = (s % P) * DCP + ch
                v0[pos] = e0[sl][ev]
                v1[pos] = e1[sl][ev]
                perm[pos] = ev
        w0 = _abrow(v0.reshape(P, DCP))[1].T
        w1 = _abrow(v1.reshape(P, DCP))[1].T
        per_core[c]["didx"] = np.concatenate(
            [_wrap_idx(w0, DCP), _wrap_idx(w1, DCP)], axis=1
        )
        per_core[c]["didxP"] = _wrap_idx(pv, DPT) if DPT else np.zeros(
            (P, 8), np.int16)
        per_core[c]["dperm"] = perm

    meta = dict(
        NCH=NCH,
        NUSE=NUSE,
        gbase=[int(v) for v in gbase],
        secbase=secbase,
        secn=[[int(v) for v in sn] for sn in secn],
        exactn=exactn,
        uses=uses,
        tile_uses=tile_uses,
        use_base=[int(v) for v in use_base],
        DCR=DCR,
        DP=DP,
        DP1=DP1,
        DS=DS,
        PEX=PEX,
        PEX1=PEX1,
        SEX=SEX,
        E1C=E1C,
        E0C1=E0C1,
        DPT=DPT,
    )
    return xb, per_core, meta


def _build_program(meta):
    NCH = meta["NCH"]
    NUSE = meta["NUSE"]
    gbase = meta["gbase"]
    secbase = meta["secbase"]
    secn = meta["secn"]
    exactn = meta["exactn"]
    uses = meta["uses"]
    tile_uses = meta["tile_uses"]
    use_base = meta["use_base"]
    DCR = meta["DCR"]
    DP = meta["DP"]
    DP1 = meta["DP1"]
    DS = meta["DS"]
    PEX = meta["PEX"]
    PEX1 = meta["PEX1"]
    SEX = meta["SEX"]
    E1C = meta["E1C"]
    E0C1 = meta["E0C1"]
    DPT = max(int(meta["DPT"]), 1)
    NG = math.ceil(NT / TG)
    DCP = sum(DCR)
    rbase = [0]
    for r in range(4):
        rbase.append(rbase[-1] + DCR[r])
    rbP = [0]
    for r in range(4):
        rbP.append(rbP[-1] + DP[r])
    rbP1 = [rbP[-1]]
    for r in range(4):
        rbP1.append(rbP1[-1] + DP1[r])
    maxg = max(gbase[g + 1] - gbase[g] for g in range(NG))
    maxg = max(maxg, DEC_GROUP)
    maxu = max(len(u) for u in uses)

    nc = bacc.Bacc("TRN2", target_bir_lowering=False, debug=False, num_devices=NC)

    x2p_ap = nc.dram_tensor("x2p", [NCH * 64, 2 * H], BF16, kind="ExternalInput").ap()
    xloc_ap = nc.dram_tensor("xloc", [P, NT * H], BF16, kind="ExternalInput").ap()
    w1_ap = nc.dram_tensor("w1b", [H, H], BF16, kind="ExternalInput").ap()
    w2p_ap = nc.dram_tensor("w2pb", [H, 4], BF16, kind="ExternalInput").ap()
    idx2_ap = nc.dram_tensor("idx2", [P, NCH * 8], I16, kind="ExternalInput").ap()
    dstl_ap = nc.dram_tensor("dstl", [P, NUSE], F32, kind="ExternalInput").ap()
    norm_ap = nc.dram_tensor("normv", [P, NUSE], F32, kind="ExternalInput").ap()
    dinv2_ap = nc.dram_tensor("dinv2", [P, NT], F32, kind="ExternalInput").ap()
    didx_ap = nc.dram_tensor("didx", [P, 2 * DCP * 8], I16, kind="ExternalInput").ap()
    didxP_ap = nc.dram_tensor("didxP", [P, DPT * 8], I16, kind="ExternalInput").ap()
    out_ap = nc.dram_tensor("out", [P, DCP * 2], F32, kind="ExternalOutput").ap()

    nocc = bool(os.environ.get("K_NOCC"))

    with tile.TileContext(nc) as tc:
        with (
            tc.tile_pool(name="persist", bufs=1) as pp,
            tc.tile_pool(name="gp", bufs=5) as gp,
            tc.tile_pool(name="ohp", bufs=3) as ohp,
            tc.tile_pool(name="small", bufs=3) as sp,
            tc.tile_pool(name="psA", bufs=4, space="PSUM") as psA,
            tc.tile_pool(name="psB", bufs=1, space="PSUM") as psB,
            tc.tile_pool(name="psC", bufs=1, space="PSUM") as psC,
            tc.tile_pool(name="dram", bufs=1, space="DRAM") as dp,
        ):
            # ---- persistent metadata in SBUF ----
            # Small tables needed for the first group go first; the big
            # L2/decode tables are issued after the first L1 streams so they
            # slot into the DMA timeline behind them.
            w1b = pp.tile([H, H], BF16)
            w2pb = pp.tile([H, 4], BF16)
            dinv2_sb = pp.tile([P, NT], F32)
            dstl_sb = pp.tile([P, NUSE], F32)
            norm_sb = pp.tile([P, NUSE], F32)
            idx2_sb = pp.tile([P, NCH * 8], I16)
            didx_sb = pp.tile([P, 2 * DCP * 8], I16)
            didxP_sb = pp.tile([P, DPT * 8], I16)
            xloc = pp.tile([P, NT * H], BF16)
            for sb, ap in (
                (dstl_sb, dstl_ap), (norm_sb, norm_ap), (dinv2_sb, dinv2_ap),
            ):
                nc.sync.dma_start(out=sb[:], in_=ap[:])

            # iota constants (values <= 127, exact in bf16)
            iota_f = pp.tile([P, P], BF16)
            nc.gpsimd.iota(iota_f[:], pattern=[[1, P]], base=0,
                           channel_multiplier=0,
                           allow_small_or_imprecise_dtypes=True)
            piota_f = pp.tile([P, 1], F32)
            nc.gpsimd.iota(piota_f[:], pattern=[[0, 1]], base=0,
                           channel_multiplier=1,
                           allow_small_or_imprecise_dtypes=True)
            # per-tile diagonal diag[p, d] = (p == d) * dinv2[p, t] (bf16);
            # builds overlap the first x2p stream (DVE is otherwise waiting)
            diag_sb = pp.tile([P, NT * P], BF16)
            for t in range(NT):
                nc.vector.tensor_scalar(
                    out=diag_sb[:, t * P : (t + 1) * P],
                    in0=iota_f[:],
                    scalar1=piota_f[:],
                    scalar2=dinv2_sb[:, t : t + 1],
                    op0=mybir.AluOpType.is_equal,
                    op1=mybir.AluOpType.mult,
                )

            y_sb = pp.tile([P, NT * 4], BF16)
            c_sb = pp.tile([P, NT * 4], BF16)
            out_sb = pp.tile([P, DCP * 2], F32)
            phases = int(os.environ.get("K_PHASES", "3"))
            ng_run = int(os.environ.get("K_GROUPS", str(NG)))
            body = int(os.environ.get("K_BODY", "4"))

            # y/c tables, (p, t)-major rows per core block (see _abrow)
            y_fullA = dp.tile([NC * AB0, H], BF16)
            y_fullB = dp.tile([NC * AB1, H], BF16)
            c_fullA = dp.tile([NC * AB0, H], BF16)
            c_fullB = dp.tile([NC * AB1, H], BF16)
            if not nocc:
                y_shardA = dp.tile([AB0, H], BF16)
                y_shardB = dp.tile([AB1, H], BF16)
                c_shardA = dp.tile([AB0, H], BF16)
                c_shardB = dp.tile([AB1, H], BF16)

            # -- replicate-and-write one table slice (tiles [t0, t1)) --
            # y/c values replicated x32 -> (p, t)-major 256B rows in the A/B
            # sub-tables; emitted as soon as the producing tiles are done so
            # the A half can AllGather (or be read) early.
            SLICES = ((0, 6), (6, 12), (12, 18), (18, 24), (24, 30),
                      (30, TA), (TA, 38), (38, 44), (44, 45), (45, 47),
                      (47, NT))

            def emit_slice(src_sb, dstA, dstB, t0, t1, eng=None):
                # only bytes [0:8) of each 256B table row are ever consumed
                # (L2 rhs slices cols 0:4; decode reads cols 0:4 of either
                # half of a paired element), so write just the 4 values per
                # row: 8B runs hit the 7ns descriptor floor (~40% cheaper
                # than streaming the replicated row) and no replication
                # copy is needed at all.  Bytes [8:256) stay garbage and are
                # gathered but never read.
                if t1 <= TA:
                    nc.sync.dma_start(
                        out=dstA[0:AB0, :].rearrange(
                            "(p t) e -> p t e", t=TA)[:, t0:t1, 0:4],
                        in_=src_sb[:, t0 * 4 : t1 * 4].rearrange(
                            "p (t f) -> p t f", f=4),
                    )
                else:
                    nc.sync.dma_start(
                        out=dstB[0:AB1, :].rearrange(
                            "(p t) e -> p t e",
                            t=NT - TA)[:, t0 - TA : t1 - TA, 0:4],
                        in_=src_sb[:, t0 * 4 : t1 * 4].rearrange(
                            "p (t f) -> p t f", f=4),
                    )

            def all_gather(shard, full):
                nc.gpsimd.collective_compute(
                    "AllGather", mybir.AluOpType.bypass,
                    replica_groups=[list(range(NC))],
                    ins=[shard[:].opt()], outs=[full[:].opt()],
                )

            # ---------------- Layer 1 ----------------
            y_slice_i = 0
            for g in range(ng_run):
                g0, g1c = gbase[g], gbase[g + 1]
                gcnt = g1c - g0
                ts = range(g * TG, min((g + 1) * TG, NT))
                ug = uses[g]
                ub = use_base[g]
                gath = gp.tile([P, maxg * H], BF16, tag="g")
                # contiguous stream of the group's pre-gathered x rows:
                # 512B pair lines -> full DMA rate, no per-edge descriptors
                nc.sync.dma_start(
                    out=gath[:, : gcnt * H].rearrange(
                        "p (c e) -> p c e", e=2 * H),
                    in_=x2p_ap[g0 * 64 : g1c * 64, :].rearrange(
                        "(c p) e -> p c e", p=P),
                )
                if g == 0:
                    # deferred loads: only what the L1 ramp needs goes
                    # between the first streams; idx2 (L2 desc-gen, ~60us
                    # out) and didx (decode, ~200us out) spread into the
                    # mid-L1 DMA idle so the early streams flow sooner
                    nc.sync.dma_start(out=w1b[:], in_=w1_ap[:])
                    nc.sync.dma_start(out=w2pb[:], in_=w2p_ap[:])
                    nc.sync.dma_start(out=xloc[:], in_=xloc_ap[:])
                elif g == 1:
                    nc.sync.dma_start(out=idx2_sb[:], in_=idx2_ap[:])
                elif g == 2:
                    nc.sync.dma_start(out=didx_sb[:], in_=didx_ap[:])
                    nc.sync.dma_start(out=didxP_sb[:], in_=didxP_ap[:])

                if body < 2:
                    continue
                # one-hot per USE: (iota == dstl) * norm -> bf16.
                # DVE is the L1 bottleneck engine; Pool is idle here (the L1
                # fetch is a plain DMA stream), so roughly a third of the
                # builds go to Pool (its tensor_scalar runs ~2x slower).
                oh = ohp.tile([P, maxu * P], BF16, tag="oh")
                for i in range(len(ug)):
                    # Pool helps DVE only in early groups: it must drain
                    # before the L2 A-gathers (queued behind it in program
                    # order) can fire at y_fullA-ready time (~group 6).
                    eng = nc.gpsimd if (i % 4 == 3 and g < 8) else nc.vector
                    eng.tensor_scalar(
                        out=oh[:, i * P : (i + 1) * P],
                        in0=iota_f[:],
                        scalar1=dstl_sb[:, ub + i : ub + i + 1],
                        scalar2=norm_sb[:, ub + i : ub + i + 1],
                        op0=mybir.AluOpType.is_equal,
                        op1=mybir.AluOpType.mult,
                    )
                o3d = oh[:, : len(ug) * P].rearrange("p (c e) -> p c e", e=P)

                if body < 3:
                    continue
                for t in ts:
                    jlist = tile_uses[g][t]
                    ut_ps = psA.tile([P, P], F32, tag="ut")
                    # self-loop diagonal term opens the accumulation
                    nc.tensor.matmul(
                        out=ut_ps[:],
                        lhsT=xloc[:, t * H : (t + 1) * H],
                        rhs=diag_sb[:, t * P : (t + 1) * P],
                        start=True,
                        stop=(len(jlist) == 0),
                    )
                    for i, u in enumerate(jlist):
                        j = ug[u][0] - g0
                        nc.tensor.matmul(
                            out=ut_ps[:],
                            lhsT=gath[:, j * H : (j + 1) * H],
                            rhs=o3d[:, u, :],
                            start=False,
                            stop=(i == len(jlist) - 1),
                        )
                    if body < 4:
                        continue
                    ut_sb = sp.tile([P, P], BF16, tag="utsb")
                    nc.scalar.copy(out=ut_sb[:], in_=ut_ps[:])
                    vt_ps = psB.tile([P, P], F32, tag="vt")
                    nc.tensor.matmul(out=vt_ps[:], lhsT=w1b[:], rhs=ut_sb[:],
                                     start=True, stop=True)
                    zt_sb = sp.tile([P, P], BF16, tag="ztsb")
                    nc.scalar.activation(out=zt_sb[:], in_=vt_ps[:],
                                         func=mybir.ActivationFunctionType.Relu)
                    y_ps = psC.tile([P, 4], F32, tag="yps")
                    nc.tensor.matmul(out=y_ps[:], lhsT=zt_sb[:], rhs=w2pb[:],
                                     start=True, stop=True)
                    nc.scalar.copy(out=y_sb[:, t * 4 : t * 4 + 4], in_=y_ps[:])

                # emit y table slices whose producing tiles are now done so
                # the A half hits DRAM (and AllGathers) mid-phase
                if body >= 4:
                    done_t = min((g + 1) * TG, NT)
                    while y_slice_i < len(SLICES) and \
                            SLICES[y_slice_i][1] <= done_t:
                        t0, t1 = SLICES[y_slice_i]
                        emit_slice(y_sb,
                                   y_fullA if nocc else y_shardA,
                                   y_fullB if nocc else y_shardB, t0, t1)
                        if t1 == TA and not nocc:
                            all_gather(y_shardA, y_fullA)
                        if t1 == NT and not nocc:
                            all_gather(y_shardB, y_fullB)
                        y_slice_i += 1

            # ---------------- Layer 2 ----------------
            # Gather issuance is decoupled from the compute loop so the Pool
            # engine (in-order) reaches the A-sub-table gathers as soon as
            # y_fullA is complete (mid-L1), filling the L1->L2 DMA seam.  The
            # B gathers wait on all of y; interleaving [A0..A3, B0, A4, B1,
            # A5, ...] keeps each A(k+4)'s buffer-reuse wait (on group k's
            # matmuls) behind the B(k) it needs -- no cyclic wait.
            l2_gath = [None] * NG

            def issue_l2_gather(g, s):
                if l2_gath[g] is None:
                    l2_gath[g] = gp.tile([P, maxg * H], BF16, tag="g",
                                         name=f"l2g_{g}")
                g0 = gbase[g]
                gcnt = gbase[g + 1] - g0
                g3d = l2_gath[g][:, : gcnt * H].rearrange(
                    "p (c e) -> p c e", e=H)
                cnt = exactn[g][s]
                if cnt == 0:
                    return
                b0 = secbase[g][s] - g0
                # the very first call is split so its (shorter) descriptor
                # generation starts the post-L1 DMA chain sooner; the second
                # half's desc-gen overlaps the first half's transfer
                parts = ((cnt // 256 * 128, True),) if (g == 0 and s == 0
                                                        and cnt > 512) else ()
                done = 0
                for cut, _ in parts:
                    nc.gpsimd.dma_gather(
                        out_ap=g3d[:, b0 + done // P : b0 + cut // P, :],
                        in_ap=(y_fullA, y_fullB)[s],
                        idxs_ap=idx2_sb[
                            :, (g0 + b0 + done // P) * 8
                            : (g0 + b0 + cut // P) * 8],
                        num_idxs=cut - done,
                        num_idxs_reg=cut - done,
                        elem_size=H,
                        single_packet=False,
                    )
                    done = cut
                hcnt = -(-cnt // P)
                nc.gpsimd.dma_gather(
                    out_ap=g3d[:, b0 + done // P : b0 + hcnt, :],
                    in_ap=(y_fullA, y_fullB)[s],
                    idxs_ap=idx2_sb[
                        :, (g0 + b0 + done // P) * 8 : (g0 + b0 + hcnt) * 8],
                    num_idxs=cnt - done,
                    num_idxs_reg=cnt - done,
                    elem_size=H,
                    single_packet=False,
                )

            if phases >= 2:
                order = []
                for g in range(min(5, ng_run)):
                    order.append((g, 0))
                for k in range(ng_run):
                    order.append((k, 1))
                    if k + 5 < ng_run:
                        order.append((k + 5, 0))
                for g, s in order:
                    issue_l2_gather(g, s)

            c_slice_i = 0
            for g in (range(ng_run) if phases >= 2 else []):
                g0, g1c = gbase[g], gbase[g + 1]
                gcnt = g1c - g0
                ts = range(g * TG, min((g + 1) * TG, NT))
                ug = uses[g]
                ub = use_base[g]
                gath = l2_gath[g]
                g3d = gath[:, : gcnt * H].rearrange("p (c e) -> p c e", e=H)

                oh = ohp.tile([P, maxu * P], BF16, tag="oh")
                for i in range(len(ug)):
                    # taper: Pool must drain before decode desc-gen (queued
                    # behind it) can overlap the last L2 transfers
                    eng = nc.gpsimd if i % 4 == 3 else nc.vector
                    eng.tensor_scalar(
                        out=oh[:, i * P : (i + 1) * P],
                        in0=iota_f[:],
                        scalar1=dstl_sb[:, ub + i : ub + i + 1],
                        scalar2=norm_sb[:, ub + i : ub + i + 1],
                        op0=mybir.AluOpType.is_equal,
                        op1=mybir.AluOpType.mult,
                    )
                o3d = oh[:, : len(ug) * P].rearrange("p (c e) -> p c e", e=P)

                for t in ts:
                    jlist = tile_uses[g][t]
                    c_ps = psC.tile([P, 4], F32, tag="cps")
                    for i, u in enumerate(jlist):
                        nc.tensor.matmul(
                            out=c_ps[:],
                            lhsT=o3d[:, u, :],
                            rhs=g3d[:, ug[u][0] - g0, 0:4],
                            start=(i == 0),
                            stop=(i == len(jlist) - 1),
                        )
                    # c = (dinv2 * y) + c_ps : fused self-loop add from PSUM
                    nc.vector.scalar_tensor_tensor(
                        out=c_sb[:, t * 4 : t * 4 + 4],
                        in0=y_sb[:, t * 4 : t * 4 + 4],
                        scalar=dinv2_sb[:, t : t + 1],
                        in1=c_ps[:],
                        op0=mybir.AluOpType.mult,
                        op1=mybir.AluOpType.add,
                    )

                done_t = min((g + 1) * TG, NT)
                while c_slice_i < len(SLICES) and SLICES[c_slice_i][1] <= done_t:
                    t0, t1 = SLICES[c_slice_i]
                    emit_slice(c_sb,
                               c_fullA if nocc else c_shardA,
                               c_fullB if nocc else c_shardB, t0, t1)
                    if t1 == TA and not nocc:
                        all_gather(c_shardA, c_fullA)
                    if t1 == NT and not nocc:
                        all_gather(c_shardB, c_fullB)
                    c_slice_i += 1

            # ---------------- Decode ----------------
            # Runs 0-3 sorted by (row(e0) in B, row(e1) in B): the e0 gather
            # uses c_fullA for runs 0-1 / c_fullB for 2-3; the e1 gather
            # alternates per run. Process DEC_GROUP chunks per slice; calls
            # split at run boundaries so each uses a single sub-table, with
            # exact num_idxs (chunk-major slot packing puts real slots in a
            # call-position prefix).
            def paired_view(table):
                # overlapping window view [row, 2 rows]: idx R fetches rows
                # R, R+1 in one 512B element (elem_step=H, elem_size=2H)
                v = table[:].copy()
                n = v.ap[0][1]
                v.ap = bass_rust.VecI64Pair([[H, n - 1], [1, 2 * H]])
                return v

            def seg_call(gt, table, k, s0_, s1_, seg0, exact, idxs, off,
                         paired=False):
                # one gather over chunk span [s0_, s1_) of a segment that
                # starts at chunk seg0 and holds `exact` real idx positions
                if s1_ <= s0_:
                    return
                if paired:
                    table = paired_view(table)
                    d0 = (s0_ - seg0) // 2
                    cnt = min(((s1_ - seg0) // 2) * P, exact) - d0 * P
                    if cnt <= 0:
                        return
                    # the first decode call's desc-gen is exposed at the
                    # L2->decode seam: emit a small head call first
                    cuts = []
                    if k == 0 and s0_ == 0 and cnt > 512:
                        cuts = [cnt // 256 * 128]
                    done = 0
                    for cut in cuts + [cnt]:
                        nd0, nd1 = done // P, -(-cut // P)
                        nc.gpsimd.dma_gather(
                            out_ap=gt[:, (s0_ - k + 2 * nd0) * H
                                      : (s0_ - k + 2 * nd1) * H]
                            .rearrange("p (c e) -> p c e", e=2 * H),
                            in_ap=table,
                            idxs_ap=idxs[:, (off + d0 + nd0) * 8
                                         : (off + d0 + nd1) * 8],
                            num_idxs=cut - done,
                            num_idxs_reg=cut - done,
                            elem_size=2 * H,
                            elem_step=H,
                            single_packet=False,
                        )
                        done = cut
                else:
                    cnt = min((s1_ - seg0) * P, exact) - (s0_ - seg0) * P
                    if cnt <= 0:
                        return
                    hcnt = -(-cnt // P)
                    nc.gpsimd.dma_gather(
                        out_ap=gt[:, (s0_ - k) * H : (s0_ - k + hcnt) * H]
                        .rearrange("p (c e) -> p c e", e=H),
                        in_ap=table,
                        idxs_ap=idxs[:, (off + s0_) * 8 : (off + s0_ + hcnt) * 8],
                        num_idxs=cnt,
                        num_idxs_reg=cnt,
                        elem_size=H,
                        single_packet=False,
                    )

            def dec_calls(gt, side, k, k1):
                # run layout: [2*DP e0-paired][2*DP1 e1-paired][DS singles]
                for r in range(4):
                    rb = rbase[r]
                    m0 = rb + 2 * DP[r]
                    m1 = m0 + 2 * DP1[r]
                    end = rbase[r + 1]
                    if side == 0:
                        table = (c_fullA, c_fullB)[r // 2]
                        seg_call(gt, table, k, max(k, rb), min(k1, m0),
                                 rb, PEX[r], didxP_sb, rbP[r], paired=True)
                        seg_call(gt, table, k, max(k, m0), min(k1, m1),
                                 m0, E0C1[r], didx_sb, 0)
                        seg_call(gt, table, k, max(k, m1), min(k1, end),
                                 m1, SEX[r], didx_sb, 0)
                    else:
                        table = (c_fullA, c_fullB)[r % 2]
                        seg_call(gt, table, k, max(k, rb), min(k1, m0),
                                 rb, E1C[r], didx_sb, DCP)
                        seg_call(gt, table, k, max(k, m0), min(k1, m1),
                                 m0, PEX1[r], didxP_sb, rbP1[r], paired=True)
                        seg_call(gt, table, k, max(k, m1), min(k1, end),
                                 m1, SEX[r], didx_sb, DCP)

            if phases >= 3:
                k = 0
                windows = []
                while k < DCP:
                    w = min(DEC_GROUP, DCP - k)
                    if DCP - k <= DEC_GROUP and w > 26:
                        h2 = (w // 2) & ~1  # keep window bounds even
                        windows += [w - h2, h2]
                    else:
                        windows += [w]
                    k += w
                k = 0
                for w in windows:
                    k1 = k + w
                    gts = []
                    for side in (0, 1):
                        gt = gp.tile([P, maxg * H], BF16, tag="g")
                        dec_calls(gt, side, k, k1)
                        gts.append(gt)
                    nc.vector.tensor_tensor(
                        out=out_sb[:, k * 2 : k1 * 2].rearrange(
                            "p (c f) -> p c f", f=2),
                        in0=gts[0][:, : (k1 - k) * H].rearrange(
                            "p (c e) -> p c e", e=H)[:, :, 0:2],
                        in1=gts[1][:, : (k1 - k) * H].rearrange(
                            "p (c e) -> p c e", e=H)[:, :, 2:4],
                        op=mybir.AluOpType.add,
                    )
                    nc.sync.dma_start(out=out_ap[:, k * 2 : k1 * 2],
                                      in_=out_sb[:, k * 2 : k1 * 2])
                    k = k1

    nc.compile()
    return nc


def kernel(x, edge_index, pos_edge_index, neg_edge_index, W1, W2, Wlin):
    x = np.asarray(x, np.float32)
    W1 = np.asarray(W1, np.float32)
    W2 = np.asarray(W2, np.float32)
    Wlin = np.asarray(Wlin, np.float32)

    xb, per_core, meta = _preprocess(
        x, edge_index, pos_edge_index, neg_edge_index
    )

    # fold W2 and Wlin: cols 0,1 pair with e0 (Wlin[:, :128]), cols 2,3 with e1
    Wl = np.stack([Wlin[0, :H], Wlin[1, :H], Wlin[0, H:], Wlin[1, H:]], axis=1)
    W2p = (W2 @ Wl).astype(np.float32)

    nc = _build_program(meta)

    w1b = W1.astype(BF)
    w2pb = W2p.astype(BF)
    DCP = sum(meta["DCR"])
    perms = []
    in_maps = []
    for c in range(NC):
        m = dict(per_core[c])
        perms.append(m.pop("dperm"))
        m["w1b"] = w1b
        m["w2pb"] = w2pb
        in_maps.append(m)

    res = run_bass_kernel_spmd(nc, in_maps, core_ids=list(range(NC)))

    out = np.empty((NE_EVAL, 2), np.float32)
    for c in range(NC):
        shard = res.results[c]["out"].reshape(DCP * P, 2)  # row = p*DCP + k
        perm = perms[c]
        valid = perm >= 0
        out[c * EV_PC + perm[valid]] = shard[valid]
    return out
